# revision 3
# baseline (speedup 1.0000x reference)
"""ChebConv(K=3) x2 + BN GNN kernel for 8 Trainium2 NeuronCores.

Strategy:
  - Nodes dst-sharded across 8 cores (12500 each, padded to 12544 = 98*128).
  - ChebNet algebra refactored: out = x@(W0-W2) + L(x@W1) + L(L(x@(2W2)))
    with L = -D^-1/2 A D^-1/2 applied as: scale rows by dis on the way into
    the gather table, scale by -dis on the way out of the aggregation.
  - Each propagate: per dst-tile (128 dsts), gather source rows from a
    replicated table in HBM via dma_gather (int16 idx => 4 row-chunks of
    <=32768), build a 0/1 selection matrix on DVE (iota==dstloc), and
    aggregate with TensorE matmuls accumulating in PSUM.
  - Tables replicated across cores via AllGather after each half-step.
  - Layer-1 tables fp16 (256B rows), layer-2 tables fp32 (64 feats = 256B).
"""
import sys
import time

for _p in ("/opt/trn_rl_repo",):
    if _p not in sys.path:
        sys.path.insert(0, _p)

import numpy as np

import concourse.bass as bass
import concourse.bacc as bacc
import concourse.mybir as mybir
import concourse.tile as tile
from concourse.masks import make_identity

N_CORES = 8
EPS = 1e-5
CHUNK = 32768  # dma_gather int16 index limit per chunk
NI_MAX_BATCHES = 8  # <=1024 idxs per dma_gather (HW ring limit)


def make_cfg(N, DIN, HID, OUT):
    SH = N // N_CORES
    assert SH * N_CORES == N
    TILES = (SH + 127) // 128
    SHP = TILES * 128
    TR = N_CORES * SHP  # table rows
    NCH = (TR + CHUNK - 1) // CHUNK
    return dict(N=N, DIN=DIN, HID=HID, OUT=OUT, SH=SH, SHP=SHP, TILES=TILES,
                TR=TR, NCH=NCH)


CFG = make_cfg(100000, 128, 128, 64)

# ---------------------------------------------------------------------------
# Host preprocessing
# ---------------------------------------------------------------------------


def preprocess_edges(edge_index, cfg):
    """Group edges by (dst shard, dst tile, src chunk); build shared program
    meta (per-(tile,chunk) batch counts = max over cores) and per-core int16
    gather-index + dstloc streams."""
    N, SH, SHP, TILES, NCH = cfg["N"], cfg["SH"], cfg["SHP"], cfg["TILES"], cfg["NCH"]
    src = edge_index[0].astype(np.int64)
    dst = edge_index[1].astype(np.int64)

    deg = np.bincount(src, minlength=N).astype(np.float64)
    dis = np.where(deg > 0, 1.0 / np.sqrt(np.maximum(deg, 1.0)), 0.0).astype(np.float32)

    shard = dst // SH
    tloc = (dst % SH) // 128
    rloc = (dst % SH) % 128
    rowof_all = (np.arange(N) // SH) * SHP + (np.arange(N) % SH)
    rowof = rowof_all[src]
    chunk = rowof // CHUNK

    key = (shard * TILES + tloc) * NCH + chunk
    order = np.argsort(key, kind="stable")
    cnt = np.bincount(key, minlength=N_CORES * TILES * NCH).reshape(N_CORES, TILES, NCH)
    nb = -(-cnt // 128)  # ceil
    nb_shared = nb.max(axis=0)  # [TILES, NCH]
    nb_shared[:, 0] = np.maximum(nb_shared[:, 0], 1)  # ensure PSUM init

    B_total = int(nb_shared.sum())
    S_total = B_total * 128

    # slot offsets per (tile, chunk) in the shared stream
    flat_nb = nb_shared.reshape(-1)
    slot_off = np.zeros(TILES * NCH + 1, np.int64)
    np.cumsum(flat_nb * 128, out=slot_off[1:])

    idx_stream = np.zeros((N_CORES, S_total), np.int16)
    dloc_stream = np.full((N_CORES, S_total), 255, np.int16)

    # per-core group start offsets in sorted edge order
    grp_cnt = np.bincount(key, minlength=N_CORES * TILES * NCH)
    grp_start = np.zeros(N_CORES * TILES * NCH + 1, np.int64)
    np.cumsum(grp_cnt, out=grp_start[1:])

    src_local = (rowof % CHUNK).astype(np.int16)
    s_sorted = src_local[order]
    r_sorted = rloc[order].astype(np.int16)

    for c in range(N_CORES):
        for t in range(TILES):
            for ch in range(NCH):
                g = (c * TILES + t) * NCH + ch
                n = grp_cnt[g]
                if n == 0:
                    continue
                a = grp_start[g]
                o = slot_off[t * NCH + ch]
                idx_stream[c, o:o + n] = s_sorted[a:a + n]
                dloc_stream[c, o:o + n] = r_sorted[a:a + n]

    # wrapped idx layout: slot s -> [s%16, s//16], replicated to 128 partitions
    idx_w = idx_stream.reshape(N_CORES, S_total // 16, 16).transpose(0, 2, 1)
    idx_w = np.ascontiguousarray(np.tile(idx_w, (1, 8, 1)))
    # dloc tile: batch b column, partition = slot within batch
    dloc_t = dloc_stream.reshape(N_CORES, B_total, 128).transpose(0, 2, 1)
    dloc_t = np.ascontiguousarray(dloc_t).astype(np.float32)

    meta = dict(nb_shared=nb_shared, B_total=B_total, S_total=S_total,
                slot_off=slot_off)
    return meta, dis, idx_w, dloc_t


def build_host_inputs(x, dis, weights, cfg):
    """Per-core input tensors (excluding idx/dloc)."""
    (W1, b1, W2, b2, g1, beta1, m1, v1, g2, beta2, m2, v2) = weights
    N, SH, SHP, TILES, TR = cfg["N"], cfg["SH"], cfg["SHP"], cfg["TILES"], cfg["TR"]
    DIN, HID, OUT = cfg["DIN"], cfg["HID"], cfg["OUT"]

    Wcat1 = np.concatenate([W1[0] - W1[2], W1[1], 2.0 * W1[2]], axis=1).astype(np.float16)
    Wcat2 = np.concatenate([W2[0] - W2[2], W2[1], 2.0 * W2[2]], axis=1).astype(np.float16)
    A1 = (g1 / np.sqrt(v1 + EPS)).astype(np.float32)
    C1 = (beta1 + (b1 - m1) * A1).astype(np.float32)
    A2 = (g2 / np.sqrt(v2 + EPS)).astype(np.float32)
    C2 = (beta2 + (b2 - m2) * A2).astype(np.float32)
    AC1 = np.concatenate([np.tile(A1, (128, 1)), np.tile(C1, (128, 1))], axis=1)
    AC2 = np.concatenate([np.tile(A2, (128, 1)), np.tile(C2, (128, 1))], axis=1)

    tab0 = np.zeros((TR, DIN), np.float16)
    xp = np.zeros((N_CORES, SHP, DIN), np.float32)
    disp = np.zeros((N_CORES, SHP), np.float32)
    for c in range(N_CORES):
        xs = x[c * SH:(c + 1) * SH]
        xp[c, :SH] = xs
        disp[c, :SH] = dis[c * SH:(c + 1) * SH]
        tab0[c * SHP:c * SHP + SH] = (dis[c * SH:(c + 1) * SH, None] * xs).astype(np.float16)

    in_maps = []
    for c in range(N_CORES):
        d = disp[c].reshape(TILES, 128).T  # [128, TILES]
        in_maps.append({
            "xT": np.ascontiguousarray(xp[c].T).astype(np.float16),
            "tab0": tab0,
            "disP": np.ascontiguousarray(d),
            "disN": np.ascontiguousarray(-d),
            "dis2N": np.ascontiguousarray(-(d.astype(np.float64) ** 2)).astype(np.float32),
            "Wcat1": Wcat1,
            "Wcat2": Wcat2,
            "AC1": AC1.astype(np.float32),
            "AC2": AC2.astype(np.float32),
        })
    return in_maps


# ---------------------------------------------------------------------------
# Bass program
# ---------------------------------------------------------------------------


def build_program(cfg, meta):
    dt = mybir.dt
    f16, f32, i16 = dt.float16, dt.float32, dt.int16
    SHP, TILES, TR, NCH = cfg["SHP"], cfg["TILES"], cfg["TR"], cfg["NCH"]
    DIN, HID, OUT = cfg["DIN"], cfg["HID"], cfg["OUT"]
    nb_shared = meta["nb_shared"]
    B_total, S_total = meta["B_total"], meta["S_total"]
    slot_off = meta["slot_off"]

    nc = bacc.Bacc("TRN2", target_bir_lowering=False, debug=False,
                   num_devices=N_CORES, num_swdge_queues=4)

    xT_d = nc.dram_tensor("xT", [128, SHP], f16, kind="ExternalInput")
    tab0_d = nc.dram_tensor("tab0", [TR, DIN], f16, kind="ExternalInput")
    disP_d = nc.dram_tensor("disP", [128, TILES], f32, kind="ExternalInput")
    disN_d = nc.dram_tensor("disN", [128, TILES], f32, kind="ExternalInput")
    dis2N_d = nc.dram_tensor("dis2N", [128, TILES], f32, kind="ExternalInput")
    W1_d = nc.dram_tensor("Wcat1", [DIN, 3 * HID], f16, kind="ExternalInput")
    W2_d = nc.dram_tensor("Wcat2", [HID, 3 * OUT], f16, kind="ExternalInput")
    AC1_d = nc.dram_tensor("AC1", [128, 2 * HID], f32, kind="ExternalInput")
    AC2_d = nc.dram_tensor("AC2", [128, 2 * OUT], f32, kind="ExternalInput")
    idx_d = nc.dram_tensor("idxs", [128, S_total // 16], i16, kind="ExternalInput")
    dloc_d = nc.dram_tensor("dloc", [128, B_total], f32, kind="ExternalInput")
    out_d = nc.dram_tensor("out", [SHP, OUT], f32, kind="ExternalOutput")

    rg = [list(range(N_CORES))]

    with tile.TileContext(nc) as tc:
        import contextlib
        ctx = contextlib.ExitStack()
        with ctx:
            const_p = ctx.enter_context(tc.tile_pool(name="const", bufs=1))
            big_p = ctx.enter_context(tc.tile_pool(name="big", bufs=1))
            idx_p = ctx.enter_context(tc.tile_pool(name="idx", bufs=4))
            g16_p = ctx.enter_context(tc.tile_pool(name="g16", bufs=6))
            g32_p = ctx.enter_context(tc.tile_pool(name="g32", bufs=6))
            s_p = ctx.enter_context(tc.tile_pool(name="sel", bufs=6))
            ev_p = ctx.enter_context(tc.tile_pool(name="ev", bufs=4))
            ps_prop = ctx.enter_context(tc.tile_pool(name="psprop", bufs=4, space="PSUM"))
            ps_dense = ctx.enter_context(tc.tile_pool(name="psdense", bufs=2, space="PSUM"))
            ps_tr = ctx.enter_context(tc.tile_pool(name="pstr", bufs=2, space="PSUM"))
            dram_p = ctx.enter_context(tc.tile_pool(name="dram", bufs=1, space="DRAM"))

            # ---- constants ----
            iota_sb = const_p.tile([128, 128], f32)
            nc.gpsimd.iota(iota_sb[:], pattern=[[1, 128]], base=0,
                           channel_multiplier=0,
                           allow_small_or_imprecise_dtypes=True)
            ident = const_p.tile([128, 128], f16)
            make_identity(nc, ident[:])
            W1_sb = const_p.tile([DIN, 3 * HID], f16)
            nc.sync.dma_start(out=W1_sb[:], in_=W1_d.ap())
            W2_sb = const_p.tile([HID, 3 * OUT], f16)
            nc.sync.dma_start(out=W2_sb[:], in_=W2_d.ap())
            AC1_sb = const_p.tile([128, 2 * HID], f32)
            nc.sync.dma_start(out=AC1_sb[:], in_=AC1_d.ap())
            AC2_sb = const_p.tile([128, 2 * OUT], f32)
            nc.sync.dma_start(out=AC2_sb[:], in_=AC2_d.ap())
            disP_sb = const_p.tile([128, TILES], f32)
            nc.sync.dma_start(out=disP_sb[:], in_=disP_d.ap())
            disN_sb = const_p.tile([128, TILES], f32)
            nc.sync.dma_start(out=disN_sb[:], in_=disN_d.ap())
            dis2N_sb = const_p.tile([128, TILES], f32)
            nc.sync.dma_start(out=dis2N_sb[:], in_=dis2N_d.ap())
            dloc_sb = const_p.tile([128, B_total], f32)
            nc.sync.dma_start(out=dloc_sb[:], in_=dloc_d.ap())

            # ---- big resident arrays ----
            xT_sb = big_p.tile([128, SHP], f16, tag="xT")
            nc.sync.dma_start(out=xT_sb[:], in_=xT_d.ap())
            za_sb = big_p.tile([128, TILES * HID], f16, tag="za")
            zb_sb = big_p.tile([128, TILES * HID], f16, tag="zb")

            # ---- DRAM bounce + tables ----
            b1_t = dram_p.tile([SHP, HID], f16)
            t1_t = dram_p.tile([TR, HID], f16)
            b2_t = dram_p.tile([SHP, HID], f16)
            t2_t = dram_p.tile([TR, HID], f16)
            b3_t = dram_p.tile([SHP, OUT], f32)
            t3_t = dram_p.tile([TR, OUT], f32)
            b4_t = dram_p.tile([SHP, OUT], f32)
            t4_t = dram_p.tile([TR, OUT], f32)

            gq = [0]  # rotating gather queue

            def dense(lhs_sb, W_sb, F, za_dst, zb_dst, bounce, ev_dtype):
                """z = lhs.T @ [Wa|Wb|Wc]; za kept, zb=dis*z_b kept, z_c=dis*z_c -> bounce."""
                for t in range(TILES):
                    lhsT = lhs_sb[:, t * 128:(t + 1) * 128]
                    ps = ps_dense.tile([128, 3 * F], f32)
                    for j in range(3):
                        nc.tensor.matmul(ps[:, j * F:(j + 1) * F], lhsT,
                                         W_sb[:, j * F:(j + 1) * F],
                                         start=True, stop=True)
                    nc.vector.tensor_copy(za_dst[:, t * F:(t + 1) * F], ps[:, 0:F])
                    nc.vector.tensor_scalar(zb_dst[:, t * F:(t + 1) * F],
                                            ps[:, F:2 * F], disP_sb[:, t:t + 1],
                                            None, mybir.AluOpType.mult)
                    zc = ev_p.tile([128, 3 * OUT if F == OUT else F], ev_dtype, tag="zc")
                    nc.vector.tensor_scalar(zc[:, :F], ps[:, 2 * F:3 * F],
                                            disP_sb[:, t:t + 1], None,
                                            mybir.AluOpType.mult)
                    nc.sync.dma_start(out=bounce[t * 128:(t + 1) * 128, :],
                                      in_=zc[:, :F])

            def propagate(table, F, gdt, g_pool, evac):
                """y[dst] = sum_e table[src_e]; evac(t, psum) consumes PSUM."""
                for t in range(TILES):
                    ps = ps_prop.tile([128, F], f32)
                    total_nb = int(nb_shared[t].sum())
                    done = 0
                    for ch in range(NCH):
                        nbc = int(nb_shared[t, ch])
                        if nbc == 0:
                            continue
                        rows0 = ch * CHUNK
                        rows1 = min((ch + 1) * CHUNK, TR)
                        base = slot_off[t * NCH + ch]
                        boff = base // 128
                        sub0 = 0
                        while sub0 < nbc:
                            nb_i = min(NI_MAX_BATCHES, nbc - sub0)
                            ni = nb_i * 128
                            col0 = (base + sub0 * 128) // 16
                            ixt = idx_p.tile([128, NI_MAX_BATCHES * 8], i16, tag="ix")
                            nc.sync.dma_start(out=ixt[:, :ni // 16],
                                              in_=idx_d.ap()[:, col0:col0 + ni // 16])
                            g = g_pool.tile([128, NI_MAX_BATCHES, F], gdt, tag="g")
                            nc.gpsimd.dma_gather(
                                out_ap=g[:, :nb_i, :], in_ap=table[rows0:rows1, :],
                                idxs_ap=ixt[:, :ni // 16], num_idxs=ni,
                                num_idxs_reg=ni, elem_size=F,
                                queue_num=gq[0] % 4)
                            gq[0] += 1
                            for b in range(nb_i):
                                gb = boff + sub0 + b
                                S = s_p.tile([128, 128], gdt, tag="S")
                                nc.vector.tensor_scalar(
                                    S[:], iota_sb[:], dloc_sb[:, gb:gb + 1], None,
                                    mybir.AluOpType.is_equal)
                                nc.tensor.matmul(ps[:], S[:], g[:, b, :],
                                                 start=(done == 0),
                                                 stop=(done == total_nb - 1))
                                done += 1
                            sub0 += nb_i
                    evac(t, ps)

            # ================= layer 1 =================
            dense(xT_sb, W1_sb, HID, za_sb, zb_sb, b1_t, np.float16 and f16)

            nc.gpsimd.collective_compute(
                "AllGather", mybir.AluOpType.bypass,
                ins=[b1_t[:, :]], outs=[t1_t[:, :]], replica_groups=rg)

            def evac_p1(t, ps):
                tmp = ev_p.tile([128, HID], f16, tag="tmp16")
                nc.vector.tensor_scalar(tmp[:], ps[:], dis2N_sb[:, t:t + 1], None,
                                        mybir.AluOpType.mult)
                v = ev_p.tile([128, HID], f16, tag="v16")
                nc.vector.tensor_tensor(out=v[:], in0=tmp[:],
                                        in1=zb_sb[:, t * HID:(t + 1) * HID],
                                        op=mybir.AluOpType.add)
                nc.sync.dma_start(out=b2_t[t * 128:(t + 1) * 128, :], in_=v[:])

            propagate(t1_t, HID, f16, g16_p, evac_p1)

            nc.gpsimd.collective_compute(
                "AllGather", mybir.AluOpType.bypass,
                ins=[b2_t[:, :]], outs=[t2_t[:, :]], replica_groups=rg)

            hT_sb = big_p.tile([128, SHP], f16, tag="xT")  # reuse xT slot

            def evac_p2(t, ps):
                s1 = ev_p.tile([128, HID], f32, tag="s1")
                nc.vector.tensor_scalar(s1[:], ps[:], disN_sb[:, t:t + 1], None,
                                        mybir.AluOpType.mult)
                s2 = ev_p.tile([128, HID], f32, tag="s2")
                nc.vector.tensor_tensor(out=s2[:], in0=s1[:],
                                        in1=za_sb[:, t * HID:(t + 1) * HID],
                                        op=mybir.AluOpType.add)
                s3 = ev_p.tile([128, HID], f32, tag="s1")
                nc.vector.tensor_tensor(out=s3[:], in0=s2[:], in1=AC1_sb[:, :HID],
                                        op=mybir.AluOpType.mult)
                s4 = ev_p.tile([128, HID], f32, tag="s2")
                nc.vector.tensor_tensor(out=s4[:], in0=s3[:], in1=AC1_sb[:, HID:],
                                        op=mybir.AluOpType.add)
                h = ev_p.tile([128, HID], f16, tag="h")
                nc.vector.tensor_scalar(h[:], s4[:], 0.0, None,
                                        mybir.AluOpType.max)
                pst = ps_tr.tile([128, 128], f16)
                nc.tensor.transpose(out=pst[:], in_=h[:], identity=ident[:])
                nc.vector.tensor_copy(hT_sb[:, t * 128:(t + 1) * 128], pst[:])

            propagate(t2_t, HID, f16, g16_p, evac_p2)

            # ================= layer 2 =================
            za2_sb = big_p.tile([128, TILES * OUT], f32, tag="za")  # reuse
            zb2_sb = big_p.tile([128, TILES * OUT], f32, tag="zb")

            def dense2():
                for t in range(TILES):
                    lhsT = hT_sb[:, t * 128:(t + 1) * 128]
                    ps = ps_dense.tile([128, 3 * OUT], f32)
                    for j in range(3):
                        nc.tensor.matmul(ps[:, j * OUT:(j + 1) * OUT], lhsT,
                                         W2_sb[:, j * OUT:(j + 1) * OUT],
                                         start=True, stop=True)
                    nc.vector.tensor_copy(za2_sb[:, t * OUT:(t + 1) * OUT], ps[:, 0:OUT])
                    nc.vector.tensor_scalar(zb2_sb[:, t * OUT:(t + 1) * OUT],
                                            ps[:, OUT:2 * OUT], disP_sb[:, t:t + 1],
                                            None, mybir.AluOpType.mult)
                    zc = ev_p.tile([128, 3 * OUT], f32, tag="zc32")
                    nc.vector.tensor_scalar(zc[:, :OUT], ps[:, 2 * OUT:3 * OUT],
                                            disP_sb[:, t:t + 1], None,
                                            mybir.AluOpType.mult)
                    nc.sync.dma_start(out=b3_t[t * 128:(t + 1) * 128, :],
                                      in_=zc[:, :OUT])

            dense2()

            nc.gpsimd.collective_compute(
                "AllGather", mybir.AluOpType.bypass,
                ins=[b3_t[:, :]], outs=[t3_t[:, :]], replica_groups=rg)

            def evac_p3(t, ps):
                tmp = ev_p.tile([128, OUT], f32, tag="tmp32")
                nc.vector.tensor_scalar(tmp[:], ps[:], dis2N_sb[:, t:t + 1], None,
                                        mybir.AluOpType.mult)
                v = ev_p.tile([128, OUT], f32, tag="v32")
                nc.vector.tensor_tensor(out=v[:], in0=tmp[:],
                                        in1=zb2_sb[:, t * OUT:(t + 1) * OUT],
                                        op=mybir.AluOpType.add)
                nc.sync.dma_start(out=b4_t[t * 128:(t + 1) * 128, :], in_=v[:])

            propagate(t3_t, OUT, f32, g32_p, evac_p3)

            nc.gpsimd.collective_compute(
                "AllGather", mybir.AluOpType.bypass,
                ins=[b4_t[:, :]], outs=[t4_t[:, :]], replica_groups=rg)

            def evac_p4(t, ps):
                o1 = ev_p.tile([128, OUT], f32, tag="o1")
                nc.vector.tensor_scalar(o1[:], ps[:], disN_sb[:, t:t + 1], None,
                                        mybir.AluOpType.mult)
                o2 = ev_p.tile([128, OUT], f32, tag="o2")
                nc.vector.tensor_tensor(out=o2[:], in0=o1[:],
                                        in1=za2_sb[:, t * OUT:(t + 1) * OUT],
                                        op=mybir.AluOpType.add)
                o3 = ev_p.tile([128, OUT], f32, tag="o1")
                nc.vector.tensor_tensor(out=o3[:], in0=o2[:], in1=AC2_sb[:, :OUT],
                                        op=mybir.AluOpType.mult)
                o4 = ev_p.tile([128, OUT], f32, tag="o2")
                nc.vector.tensor_tensor(out=o4[:], in0=o3[:], in1=AC2_sb[:, OUT:],
                                        op=mybir.AluOpType.add)
                nc.sync.dma_start(out=out_d.ap()[t * 128:(t + 1) * 128, :], in_=o4[:])

            propagate(t4_t, OUT, f32, g32_p, evac_p4)

    nc.compile()
    return nc


# ---------------------------------------------------------------------------
# SPMD runner (axon / PJRT path), kept warm across calls
# ---------------------------------------------------------------------------


class SpmdRunner:
    def __init__(self, nc, n_cores=N_CORES):
        import jax
        from jax.sharding import Mesh, PartitionSpec, NamedSharding
        from jax.experimental.shard_map import shard_map
        from concourse.bass2jax import (_bass_exec_p, partition_id_tensor,
                                        install_neuronx_cc_hook)
        install_neuronx_cc_hook()
        self.jax = jax
        self.n_cores = n_cores
        partition_name = nc.partition_id_tensor.name if nc.partition_id_tensor else None
        in_names, out_names, out_avals, zero_outs = [], [], [], []
        for alloc in nc.m.functions[0].allocations:
            if not isinstance(alloc, mybir.MemoryLocationSet):
                continue
            name = alloc.memorylocations[0].name
            if alloc.kind == "ExternalInput":
                if name != partition_name:
                    in_names.append(name)
            elif alloc.kind == "ExternalOutput":
                out_names.append(name)
                shape = tuple(alloc.tensor_shape)
                dtype = mybir.dt.np(alloc.dtype)
                out_avals.append(jax.core.ShapedArray(shape, dtype))
                zero_outs.append(np.zeros(shape, dtype))
        self.in_names, self.out_names = in_names, out_names
        self.out_avals, self.zero_outs = out_avals, zero_outs
        all_in_names = list(in_names) + list(out_names)
        if partition_name is not None:
            all_in_names.append(partition_name)

        def _body(*args):
            operands = list(args)
            if partition_name is not None:
                operands.append(partition_id_tensor())
            outs = _bass_exec_p.bind(
                *operands,
                out_avals=tuple(out_avals),
                in_names=tuple(all_in_names),
                out_names=tuple(out_names),
                lowering_input_output_aliases=(),
                sim_require_finite=True,
                sim_require_nnan=True,
                nc=nc,
            )
            return tuple(outs)

        devices = jax.devices()[:n_cores]
        self.mesh = Mesh(np.asarray(devices), ("core",))
        spec = PartitionSpec("core")
        self.sharding = NamedSharding(self.mesh, spec)
        in_specs = (spec,) * (len(in_names) + len(out_names))
        out_specs = (spec,) * len(out_names)
        self.fn = jax.jit(
            shard_map(_body, mesh=self.mesh, in_specs=in_specs,
                      out_specs=out_specs, check_rep=False),
            keep_unused=True,
        )

    def stage(self, in_maps):
        concat_in = [
            np.concatenate([np.asarray(in_maps[c][n]) for c in range(self.n_cores)], axis=0)
            for n in self.in_names
        ]
        concat_zeros = [
            np.zeros((self.n_cores * z.shape[0], *z.shape[1:]), z.dtype)
            for z in self.zero_outs
        ]
        dev = [self.jax.device_put(a, self.sharding) for a in concat_in + concat_zeros]
        self.jax.block_until_ready(dev)
        return dev

    def run(self, staged):
        out = self.fn(*staged)
        self.jax.block_until_ready(out)
        return out

    def unpack(self, out_arrs):
        res = []
        for c in range(self.n_cores):
            d = {}
            for i, n in enumerate(self.out_names):
                d[n] = np.asarray(out_arrs[i]).reshape(
                    self.n_cores, *self.out_avals[i].shape)[c]
            res.append(d)
        return res


_CACHE = {}


def _get_runner(cfg, meta):
    key = (tuple(sorted(cfg.items())), meta["nb_shared"].tobytes())
    if key not in _CACHE:
        nc = build_program(cfg, meta)
        _CACHE[key] = SpmdRunner(nc)
    return _CACHE[key]


def run_model(x, edge_index, weights, cfg):
    meta, dis, idx_w, dloc_t = preprocess_edges(edge_index, cfg)
    in_maps = build_host_inputs(x, dis, weights, cfg)
    for c in range(N_CORES):
        in_maps[c]["idxs"] = idx_w[c]
        in_maps[c]["dloc"] = dloc_t[c]
    r = _get_runner(cfg, meta)
    staged = r.stage(in_maps)
    res = r.unpack(r.run(staged))
    N, SH, OUT = cfg["N"], cfg["SH"], cfg["OUT"]
    out = np.empty((N, OUT), np.float32)
    for c in range(N_CORES):
        out[c * SH:(c + 1) * SH] = res[c]["out"][:SH]
    return out


def kernel(x, edge_index, W1, b1, W2, b2, g1, beta1, m1, v1, g2, beta2, m2, v2):
    x = np.asarray(x, np.float32)
    edge_index = np.asarray(edge_index)
    weights = tuple(np.asarray(w, np.float32) for w in
                    (W1, b1, W2, b2, g1, beta1, m1, v1, g2, beta2, m2, v2))
    return run_model(x, edge_index, weights, CFG)
